# revision 2
# baseline (speedup 1.0000x reference)
"""Trainium2 Bass kernel for a char-LSTM (nn_CharsLstm) — fp8 DoubleRow version.

Reference computation (B=4096 words, T=30 chars, D=512 emb, H=1024 hidden,
V=128 chars):
    xe = emb[x]                        # [B, T, D]
    scan over t: gates = xt @ W_ih.T + b_ih + h @ W_hh.T + b_hh
                 i, f, g, o = split(gates, 4)
                 c = sig(f)*c + sig(i)*tanh(g); h = sig(o)*tanh(c)
    return h                           # [B, H]

Strategy:
  - Data parallel: batch 4096 -> 8 cores x 512 words. No collectives.
  - Host folds embedding + input projection + both biases into one table:
        Wc = W_ih @ emb.T + (b_ih + b_hh)[:, None]    # [4H, V] = [4096, 128]
    so the x-path per step is a one-hot matmul with K=V=128.
  - All matmuls run in fp8 (e4m3) with MatmulPerfMode.DoubleRow: one
    instruction contracts K=256 (two 128-slices) at 0.5 PE cycles per output
    column — 2-4x the fp16 MAC rate. Per output tile [128, 512] per step:
    5 DoubleRow MMs (1 x-pair + 4 h-pairs) instead of 9 fp16 MMs.
  - fp8 precision handling (keeps rel err ~6e-3, threshold 2e-2):
      * all weights scaled x16 so W_hh entries (+-1/32) land in e4m3's
        normal range (min normal 2^-6); the 1/16 descale is folded into the
        activation instructions' free `scale` operand.
      * the x-pair's two weight slices are an error-feedback split
        w0 = fp8(16Wc), w1 = fp8(16Wc - w0) multiplying a duplicated one-hot
        moving pair — x-path quantization error ~ fp16 quality for free.
      * PSUM accumulation, c-state and all elementwise math stay fp32; only
        h is quantized (to e4m3) as the next step's moving operand.
  - Gates are reordered [i, f, o, g] so one PSUM tile [128, 4x512] holds all
    four gates of a row-chunk with i,f,o adjacent: a single Sigmoid
    activation covers [128, 1536] (3 PSUM banks), then Tanh on g. With fp8
    matmuls the ACT engine (1 elem/lane/cycle @ 1.2 GHz, 5 LUT passes of
    H per word per step) is the expected bottleneck, so the elementwise
    stage is split: ACT does the 3 LUT passes, DVE does sig(i)*tanh(g) and
    h = sig(o)*tanh(c), Pool (gpsimd) does sig(f)*c and the c update add.
"""

import numpy as np
import ml_dtypes

import concourse.bacc as bacc
import concourse.mybir as mybir
import concourse.tile as tile
from concourse.bass_utils import run_bass_kernel_spmd

B, T, D, H, V = 4096, 30, 512, 1024, 128
NCORES = 8
N = B // NCORES          # batch per core (matmul moving free dim)
KC = H // 128            # 8 h-chunks of 128 rows
NPAIR = 1 + KC // 2      # 5 DoubleRow pairs: x-pair + 4 h-pairs
F32 = mybir.dt.float32
BF16 = mybir.dt.bfloat16
FP8 = mybir.dt.float8e4
FP8NP = ml_dtypes.float8_e4m3
SIG = mybir.ActivationFunctionType.Sigmoid
TANH = mybir.ActivationFunctionType.Tanh
DR = mybir.MatmulPerfMode.DoubleRow
GSCALE = 16.0            # weights carry x16; ACT descales gates by 1/16

_cached = {}


def build_kernel(n_steps=T, repeat=1):
    nc = bacc.Bacc("TRN2", target_bir_lowering=False)

    # Host-prepared layouts (gate rows permuted to [i, f, o, g] order):
    #  wcomb [128, NPAIR*2*4096] fp8:
    #    pair 0:  wcomb[v, 0, r, m] = fp8 split of 16*Wc[m, v] (r=0 main,
    #             r=1 error-feedback residual)
    #    pair a:  wcomb[p, a, r, m] = fp8(16*W_hh[m, (2(a-1)+r)*128 + p])
    #  ohp  [128, T*2*N] fp8 : ohp[v, t, r, b] = (x[b, t] == v), r=0,1 dup
    #  h0t  [128, KC*N]  fp8 : h0t[p, k*N+b] = h0[b, k*128+p]
    #  c0t  [128, KC*N]  f32 : same layout
    #  out  [128, KC*N]  f32 : same layout (host inverts)
    wcomb_d = nc.dram_tensor("wcomb", [128, NPAIR * 2 * 4096], FP8,
                             kind="ExternalInput")
    ohp_d = nc.dram_tensor("ohp", [128, n_steps * 2 * N], FP8,
                           kind="ExternalInput")
    h0_d = nc.dram_tensor("h0t", [128, KC * N], FP8, kind="ExternalInput")
    c0_d = nc.dram_tensor("c0t", [128, KC * N], F32, kind="ExternalInput")
    out_d = nc.dram_tensor("out", [128, KC * N], F32, kind="ExternalOutput")

    with tile.TileContext(nc) as tc:
        with (
            tc.tile_pool(name="weights", bufs=1) as wpool,
            tc.tile_pool(name="state", bufs=2) as spool,
            tc.tile_pool(name="tmps", bufs=4) as tpool,
            # two half-chunk PSUM pools (2 banks each): sigma over [i,f]
            # starts while the [o,g] matmuls still run, and PSUM recycles at
            # half-chunk granularity
            tc.tile_pool(name="psum_if", bufs=2, space="PSUM") as pool_if,
            tc.tile_pool(name="psum_og", bufs=2, space="PSUM") as pool_og,
        ):
            # DMA emission order = consumption order: step-0 x-pair needs
            # wcomb pair 0 + ohp[0]; the h-pairs need h0 + wcomb pairs 1-4 in
            # order; ct is consumed by the first elementwise.
            wcomb = wpool.tile([128, NPAIR, 2, 4096], FP8, tag="wcomb")
            nc.sync.dma_start(out=wcomb[:, 0, :, :], in_=wcomb_d[:, 0:8192])
            ht = spool.tile([128, KC, N], FP8, tag="ht")
            nc.sync.dma_start(out=ht[:, :, :], in_=h0_d[:, :])
            ohp = wpool.tile([128, n_steps, 2, N], FP8, tag="ohp")
            n_oh_dma = min(4, n_steps)
            bounds = [n_steps * k // n_oh_dma for k in range(n_oh_dma + 1)]
            for lo, hi in zip(bounds, bounds[1:]):
                nc.sync.dma_start(out=ohp[:, lo:hi, :, :],
                                  in_=ohp_d[:, lo * 2 * N:hi * 2 * N])
            for a in range(1, NPAIR):
                nc.sync.dma_start(out=wcomb[:, a, :, :],
                                  in_=wcomb_d[:, a * 8192:(a + 1) * 8192])
            # per-chunk c tiles: separate tiles make the per-j independence
            # explicit (no shared-tile hazards between chunks or steps)
            cts = []
            for j in range(KC):
                ct_j = wpool.tile([128, 512], F32, tag=f"ct{j}",
                                  name=f"ct{j}")
                nc.sync.dma_start(out=ct_j, in_=c0_d[:, j * N:(j + 1) * N])
                cts.append(ct_j)

            ht_fin = wpool.tile([128, KC, N], F32, tag="ht_fin")

            # stage2(j) = tanh(c_new) + h-mul, deferred TWO chunks behind
            # stage1 (queue carries across steps) so a tanh(c) never sits at
            # the head of the ACT queue waiting on its DVE/Pool c-update —
            # that head-of-line block would delay the next step's sigmoids
            # and stall PSUM recycling for the PE.
            pend = []

            def emit_stage2(args):
                j, s_o, h_dst, dma_j = args
                t_c = tpool.tile([128, 512], BF16, tag="t_c")
                nc.scalar.activation(out=t_c, in_=cts[j], func=TANH)
                nc.vector.tensor_mul(h_dst, s_o, t_c)
                if dma_j is not None:
                    # stream each finished chunk out while the remaining
                    # chunks still compute
                    nc.sync.dma_start(out=out_d[:, dma_j * N:(dma_j + 1) * N],
                                      in_=h_dst)

            total = n_steps * repeat
            for s in range(total):
                t = s % n_steps
                last = s == total - 1
                ht_next = None if last else spool.tile([128, KC, N], FP8,
                                                       tag="ht")

                def emit_mms(pt, j, qs, a_list, t=t):
                    # pair-major: x-pair first (static), then h-pairs in k
                    # order so the last-produced h chunks are needed latest
                    for a in a_list:
                        mov = (ohp[:, t, :, :] if a == 0
                               else ht[:, 2 * (a - 1):2 * a, :])
                        for qi, q in enumerate(qs):
                            m0 = q * H + j * 128
                            nc.tensor.matmul(
                                pt[:, qi, :],
                                wcomb[:, a, :, m0:m0 + 128],
                                mov,
                                start=(a == 0), stop=(a == NPAIR - 1),
                                perf_mode=DR,
                            )

                def emit_sig(pt, s4, lo):
                    # sigmoid over a 2-gate half tile. Gate order [i,f,o,g]
                    # with the g rows carrying an extra x2 in the weights, so
                    # the g quarter yields sigmoid(2g) and tanh(g) =
                    # 2*sigmoid(2g) - 1 is recovered with a cheap DVE dual-op
                    nc.scalar.activation(out=s4[:, lo:lo + 2, :], in_=pt,
                                         func=SIG, scale=1.0 / GSCALE)

                def emit_chunk(j, front=None):
                    if front is None:
                        ptif = pool_if.tile([128, 2, 512], F32, tag="pif",
                                            name=f"pif_{s}_{j}")
                        ptog = pool_og.tile([128, 2, 512], F32, tag="pog",
                                            name=f"pog_{s}_{j}")
                        emit_mms(ptif, j, (0, 1), range(NPAIR))
                    else:
                        ptif, ptog = front
                        emit_mms(ptif, j, (0, 1), [NPAIR - 1])
                    s4 = tpool.tile([128, 4, 512], BF16, tag="s4")
                    emit_sig(ptif, s4, 0)
                    m = tpool.tile([128, 512], F32, tag="m")
                    eng = nc.vector if j >= KC - 2 else nc.gpsimd
                    eng.tensor_mul(m, cts[j], s4[:, 1, :])
                    if front is None:
                        emit_mms(ptog, j, (2, 3), range(NPAIR))
                    else:
                        emit_mms(ptog, j, (2, 3), [NPAIR - 1])
                    emit_sig(ptog, s4, 2)
                    t_g = tpool.tile([128, 512], BF16, tag="t_g")
                    nc.vector.tensor_scalar(t_g, s4[:, 3, :], 2.0, -1.0,
                                            mybir.AluOpType.mult,
                                            mybir.AluOpType.add)
                    u = tpool.tile([128, 512], BF16, tag="u")
                    nc.vector.tensor_mul(u, s4[:, 0, :], t_g)
                    nc.vector.tensor_add(cts[j], m, u)
                    push_stage2(j, s4[:, 2, :])

                def push_stage2(j, s_o, last=last, ht_next=ht_next):
                    h_dst = (ht_fin if last else ht_next)[:, j, :]
                    pend.append((j, s_o, h_dst, j if last else None))
                    if len(pend) > 2:
                        emit_stage2(pend.pop(0))

                # Step boundary: the previous step's last h chunk is produced
                # ~4us after its last matmul (sigma -> c update -> tanh -> h
                # chain), but pair a=4 of the first group needs it after only
                # a few MMs. Open all four half-tiles of j=0/j=1 and
                # front-load pairs a0-a3 (32 MMs of cover) before the first
                # a4. The pending stage2s flush AFTER the front-load matmuls
                # (PE queue is independent, and a0-a3 only read h chunks 0-5)
                # but BEFORE the a4s: a4 reads h chunks 6/7, whose writers
                # are in the pending stage2s — emission order defines the
                # dependency graph, so those readers must come after the
                # writers, and a sigma(s,0) ahead of tc(s-1,7) in the
                # in-order ACT queue would deadlock.
                fr = {}
                for j in (0, 1):
                    fr[j] = (pool_if.tile([128, 2, 512], F32, tag="pif",
                                          name=f"pif_{s}_{j}"),
                             pool_og.tile([128, 2, 512], F32, tag="pog",
                                          name=f"pog_{s}_{j}"))
                    emit_mms(fr[j][0], j, (0, 1), range(NPAIR - 1))
                    emit_mms(fr[j][1], j, (2, 3), range(NPAIR - 1))
                while pend:
                    emit_stage2(pend.pop(0))
                emit_chunk(0, front=fr[0])
                emit_chunk(1, front=fr[1])
                for j in range(2, KC):
                    emit_chunk(j)
                ht = ht_next
            while pend:
                emit_stage2(pend.pop(0))

    nc.compile()
    return nc


def _prep_core_inputs(x, wcomb, h0, c0, core, n_steps=T):
    sl = slice(core * N, (core + 1) * N)
    x_c = np.asarray(x[sl])                      # [N, T] ints
    oh = (np.arange(V, dtype=np.int64)[:, None, None]
          == x_c.T[None, :n_steps, :])           # [V, T, N]
    ohp = np.broadcast_to(oh[:, :, None, :], (V, n_steps, 2, N))
    ohp = np.ascontiguousarray(ohp).reshape(V, n_steps * 2 * N).astype(FP8NP)
    h0t = np.ascontiguousarray(
        h0[sl].reshape(N, KC, 128).transpose(2, 1, 0).reshape(128, KC * N)
    ).astype(FP8NP)
    c0t = np.ascontiguousarray(
        c0[sl].reshape(N, KC, 128).transpose(2, 1, 0).reshape(128, KC * N)
    ).astype(np.float32)
    return {"wcomb": wcomb, "ohp": ohp, "h0t": h0t, "c0t": c0t}


def _prep_weights(emb, W_ih, W_hh, b_ih, b_hh):
    # gate reorder [i, f, o, g]
    perm = np.concatenate([np.arange(0, H), np.arange(H, 2 * H),
                           np.arange(3 * H, 4 * H), np.arange(2 * H, 3 * H)])
    # g rows (block 3 after reorder) carry an extra x2: the single sigmoid
    # activation then yields sigmoid(2g) there, and tanh(g) = 2*sig(2g) - 1
    gate_scale = np.repeat([1.0, 1.0, 1.0, 2.0], H)[:, None] * GSCALE
    wc = W_ih @ emb.T + (b_ih + b_hh)[:, None]           # [4H, V]
    wc = gate_scale * wc[perm]                           # [4H, V] x16 (g x32)
    whh = gate_scale * W_hh[perm]                        # [4H, H] x16 (g x32)

    # wcomb[p, pair, r, m]
    wcomb = np.zeros((128, NPAIR, 2, 4 * H), dtype=np.float32)
    w0 = np.clip(wc, -240, 240).astype(FP8NP).astype(np.float32)
    w1 = np.clip(wc - w0, -240, 240)
    wcomb[:, 0, 0, :] = w0.T                             # [V, 4H]
    wcomb[:, 0, 1, :] = w1.T
    for a in range(1, NPAIR):
        for r in range(2):
            k = 2 * (a - 1) + r
            wcomb[:, a, r, :] = whh[:, k * 128:(k + 1) * 128].T
    return np.ascontiguousarray(
        wcomb.reshape(128, NPAIR * 2 * 4096)).astype(FP8NP)


def kernel(x, emb, W_ih, W_hh, b_ih, b_hh, h0, c0, n_steps=T):
    x = np.asarray(x)
    emb = np.asarray(emb, dtype=np.float32)
    W_ih = np.asarray(W_ih, dtype=np.float32)
    W_hh = np.asarray(W_hh, dtype=np.float32)
    b_ih = np.asarray(b_ih, dtype=np.float32)
    b_hh = np.asarray(b_hh, dtype=np.float32)
    h0 = np.asarray(h0, dtype=np.float32)
    c0 = np.asarray(c0, dtype=np.float32)

    wcomb = _prep_weights(emb, W_ih, W_hh, b_ih, b_hh)

    key = n_steps
    if key not in _cached:
        _cached[key] = build_kernel(n_steps)
    nc = _cached[key]

    in_maps = [
        _prep_core_inputs(x, wcomb, h0, c0, core, n_steps)
        for core in range(NCORES)
    ]
    res = run_bass_kernel_spmd(nc, in_maps, core_ids=list(range(NCORES)))
    kernel.last_results = res

    out = np.empty((B, H), dtype=np.float32)
    for core in range(NCORES):
        ot = res.results[core]["out"]                    # [128, KC*N]
        out[core * N:(core + 1) * N] = (
            ot.reshape(128, KC, N).transpose(2, 1, 0).reshape(N, H)
        )
    return out
